# revision 9
# baseline (speedup 1.0000x reference)
"""ClockworkRNN Trainium2 kernel (Bass/Tile), data-parallel over batch on 8 cores.

Reference semantics (see problem):
  x = X @ W + b                      # (B, T, 512)
  per step t: group i (of 8, 64 units each, period 2^i) updates iff t % 2^i == 0
    upd_i = x[t, i*64:(i+1)*64] + h[:, i*64:] @ Wc_i
    h     = tanh(concat(where(update, upd_i, h_i)))    # tanh applied to ALL units
  return h after t = T-1             # (B, 512)

Active groups at step t are always a prefix 0..g, g = min(ntz(t), 7) (g=7 at t=0).

Device design (per core, B_LOC=8 batch rows):
  - State hT kept transposed in SBUF: tile (128 part = unit within chunk,
    4 chunks of 128 units, 8 batch).
  - Per step: one PSUM bank tile (128, 4, 8). Input projection enters PSUM via
    real matmuls (lhsT = W chunk, rhs = X_tT chunk) -- X is bulk-transposed on
    the PE (128x128 transposes) into streaming SBUF tiles (d, t, b).
  - Recurrence matmuls accumulate on top using host-packed 128x128 weight
    tiles; for even g the inactive upper half-chunk carries an identity block
    so tanh(PSUM) reproduces tanh(h_old) for non-updated units in the same
    single ACT instruction.
  - ACT: instr A = tanh(PSUM[0:mh+1 chunks]) -> hT (critical path);
    instr B = tanh(hT_prev[suffix chunks])   -> hT (hidden in PE window).
"""

import numpy as np

import concourse.bacc as bacc
import concourse.mybir as mybir
import concourse.tile as tile
from concourse.bass_utils import run_bass_kernel_spmd

# ---- problem constants (hardcoded per harness contract) ----
N_CORES = 8
B_FULL = 64
B_LOC = B_FULL // N_CORES  # 8
T_FULL = 2048
D_IN = 256
D_OUT = 512
BLOCK = 128  # scan steps per t-block
FP32 = mybir.dt.float32
TANH = mybir.ActivationFunctionType.Tanh


def _g_of(t: int) -> int:
    if t == 0:
        return 7
    return min((t & -t).bit_length() - 1, 7)


def pack_rec_weights(Wcs: list[np.ndarray]) -> tuple[np.ndarray, dict]:
    """Pack recurrence weights into (20, 128, 128) fp32 lhsT tiles.

    Tile (m, v, c): lhsT for PSUM out-chunk m (units 128m..128m+128),
    contraction K-chunk c (h units 128c..128c+128), variant v
    (1 = upper group 2m+1 active, 0 = pass-through identity).
    cols 0..63   -> group 2m   (always active when chunk m is touched)
    cols 64..127 -> group 2m+1 (Wc if active, identity block if pass)
    """
    tiles = []
    index = {}
    for m in range(4):
        for v in (0, 1):
            for c in range(m, 4):
                w = np.zeros((128, 128), dtype=np.float32)
                a = 2 * m
                bgrp = 2 * m + 1
                for kk in range(128):
                    k = 128 * c + kk  # global h unit index
                    if k >= 64 * a:
                        w[kk, 0:64] = Wcs[a][k - 64 * a, :]
                    if v == 1:
                        if k >= 64 * bgrp:
                            w[kk, 64:128] = Wcs[bgrp][k - 64 * bgrp, :]
                    elif c == m and kk >= 64:
                        w[kk, kk] = 1.0
                index[(m, v, c)] = len(tiles)
                tiles.append(w)
    return np.stack(tiles), index


_REC_INDEX = pack_rec_weights(
    [np.zeros(((8 - i) * 64, 64), np.float32) for i in range(8)]
)[1]


def build_program(T: int, repeat: int = 1, b_nonzero: bool = False):
    """Emit the full SPMD program; returns compiled nc."""
    assert T % BLOCK == 0
    n_blk = T // BLOCK
    nc = bacc.Bacc(
        "TRN2", target_bir_lowering=False, debug=False, num_devices=N_CORES
    )

    X_ap = nc.dram_tensor("X", [B_LOC, T, D_IN], FP32, kind="ExternalInput").ap()
    W_ap = nc.dram_tensor("W", [D_IN, D_OUT], FP32, kind="ExternalInput").ap()
    # zero-padded W columns: WZ[:, m, 0:64] = W[:, 128m:128m+64], cols 64.. = 0.
    # Used when out-chunk m's upper group is pass-through so the start=True
    # projection matmul still covers all 128 partitions (per-partition
    # has_written clear) without adding x to pass-through units.
    WZ_ap = nc.dram_tensor("WZ", [D_IN, 4, 128], FP32, kind="ExternalInput").ap()
    RW_ap = nc.dram_tensor("RW", [20, 128, 128], FP32, kind="ExternalInput").ap()
    ID_ap = nc.dram_tensor("ID", [128, 128], FP32, kind="ExternalInput").ap()
    if b_nonzero:
        BV_ap = nc.dram_tensor("BV", [1, D_OUT], FP32, kind="ExternalInput").ap()
    out_ap = nc.dram_tensor("out", [128, 4, B_LOC], FP32, kind="ExternalOutput").ap()

    with tile.TileContext(nc) as tc:
        with (
            tc.tile_pool(name="const", bufs=1) as constp,
            tc.tile_pool(name="xraw", bufs=4) as xrawp,
            tc.tile_pool(name="xt0", bufs=3) as xt0p,
            tc.tile_pool(name="xt1", bufs=3) as xt1p,
            tc.tile_pool(name="hp", bufs=4) as hp,
            tc.tile_pool(name="ps", bufs=6, space="PSUM") as psp,
            tc.tile_pool(name="pstr", bufs=2, space="PSUM") as pstrp,
        ):
            # ---- persistent weights ----
            w_sb = constp.tile([128, 2, D_OUT], FP32, tag="w_sb")
            nc.sync.dma_start(w_sb[:], W_ap.rearrange("(c p) u -> p c u", p=128))
            wz_sb = constp.tile([128, 2, 4, 128], FP32, tag="wz_sb")
            nc.sync.dma_start(wz_sb[:], WZ_ap.rearrange("(c p) m u -> p c m u", p=128))
            rw_sb = constp.tile([128, 20, 128], FP32, tag="rw_sb")
            nc.sync.dma_start(rw_sb[:], RW_ap.rearrange("n k m -> k n m"))
            id_sb = constp.tile([128, 128], FP32, tag="id_sb")
            nc.sync.dma_start(id_sb[:], ID_ap)
            if b_nonzero:
                bv_sb = constp.tile([1, D_OUT], FP32, tag="bv_sb")
                nc.sync.dma_start(bv_sb[:], BV_ap)
                ones_sb = constp.tile([1, B_LOC], FP32, tag="ones_sb")
                nc.gpsimd.memset(ones_sb[:], 1.0)

            def body(_iv=None):
                xt_cur = [None, None]
                xt_next = [None, None]
                xraw_tiles = {}

                def emit_xdma(blk: int, bb: int):
                    xr = xrawp.tile([128, D_IN], FP32, tag="xraw")
                    nc.sync.dma_start(
                        xr[:], X_ap[bb, blk * BLOCK : (blk + 1) * BLOCK, :]
                    )
                    xraw_tiles[(blk, bb)] = xr

                def emit_transpose(blk: int, pair: int):
                    bb, dc = pair // 2, pair % 2
                    if pair == 0:
                        xt_next[0] = xt0p.tile([128, BLOCK, B_LOC], FP32, tag="xt0", name="xt0")
                        xt_next[1] = xt1p.tile([128, BLOCK, B_LOC], FP32, tag="xt1", name="xt1")
                    xr = xraw_tiles[(blk, bb)]
                    ptr = pstrp.tile([128, 128], FP32, tag="pstr")
                    nc.tensor.transpose(
                        ptr[:], xr[:, dc * 128 : (dc + 1) * 128], id_sb[:]
                    )
                    nc.vector.tensor_copy(xt_next[dc][:, :, bb], ptr[:])

                def emit_step(t: int, h_prev):
                    g = _g_of(t)
                    mh = g // 2
                    ps_t = psp.tile([128, 4, B_LOC], FP32, tag="ps")
                    h_t = hp.tile([128, 4, B_LOC], FP32, tag="h")
                    t_off = t % BLOCK
                    # --- projection (+ bias) matmuls into PSUM ---
                    n_proj = (mh + 1) * (2 + (1 if b_nonzero else 0))
                    k_proj = 0
                    for m in range(mh + 1):
                        pass_chunk = g < 2 * m + 1  # only possible at m == mh
                        for dc in range(2):
                            lhsT = (
                                wz_sb[:, dc, m, :]
                                if pass_chunk
                                else w_sb[:, dc, 128 * m : 128 * m + 128]
                            )
                            nc.tensor.matmul(
                                ps_t[:, m, :],
                                lhsT,
                                xt_cur[dc][:, t_off, :],
                                start=k_proj == 0,
                                stop=(t == 0 and k_proj == n_proj - 1),
                            )
                            k_proj += 1
                        if b_nonzero:
                            # after W proj so has_written bits are already set
                            nc.tensor.matmul(
                                ps_t[0:64, m, :] if pass_chunk else ps_t[:, m, :],
                                bv_sb[0:1, 128 * m : 128 * m + (64 if pass_chunk else 128)],
                                ones_sb[:],
                                start=False,
                                stop=(t == 0 and k_proj == n_proj - 1),
                            )
                            k_proj += 1
                    # --- off-critical-path tanh of untouched suffix chunks ---
                    if mh < 3:
                        nc.scalar.activation(
                            h_t[:, mh + 1 : 4, :], h_prev[:, mh + 1 : 4, :], TANH
                        )
                    # --- recurrence matmuls ---
                    if t > 0:
                        for m in range(mh + 1):
                            v = 1 if g >= 2 * m + 1 else 0
                            for c in range(m, 4):
                                nc.tensor.matmul(
                                    ps_t[:, m, :],
                                    rw_sb[:, _REC_INDEX[(m, v, c)], :],
                                    h_prev[:, c, :],
                                    start=False,
                                    stop=(m, c) == (mh, 3),
                                )
                    # --- critical-path tanh of updated prefix ---
                    nc.scalar.activation(
                        h_t[:, 0 : mh + 1, :], ps_t[:, 0 : mh + 1, :], TANH
                    )
                    return h_t

                # prologue: block 0 loads + transposes
                for bb in range(8):
                    emit_xdma(0, bb)
                for pair in range(16):
                    emit_transpose(0, pair)
                xt_cur[0], xt_cur[1] = xt_next
                xt_next[0] = xt_next[1] = None

                h_prev = None
                for blk in range(n_blk):
                    nxt = blk + 1
                    for s in range(BLOCK):
                        t = blk * BLOCK + s
                        if nxt < n_blk:
                            if s < 8:
                                emit_xdma(nxt, s)
                            if s % 8 == 4:
                                emit_transpose(nxt, s // 8)
                        h_prev = emit_step(t, h_prev)
                    if nxt < n_blk:
                        xt_cur[0], xt_cur[1] = xt_next
                        xt_next[0] = xt_next[1] = None
                nc.sync.dma_start(out_ap, h_prev[:])

            if repeat == 1:
                body()
            else:
                tc.For_i_unrolled(0, repeat, 1, body, max_unroll=1)

    nc.compile()
    return nc


# ---- host-side entry point ----
_PROG_CACHE: dict = {}


def _get_prog(T: int, b_nonzero: bool, repeat: int = 1):
    key = (T, b_nonzero, repeat)
    if key not in _PROG_CACHE:
        _PROG_CACHE[key] = build_program(T, repeat=repeat, b_nonzero=b_nonzero)
    return _PROG_CACHE[key]


def make_in_maps(X, W, b, Wcs, b_nonzero: bool):
    X = np.ascontiguousarray(np.asarray(X, dtype=np.float32))
    W = np.ascontiguousarray(np.asarray(W, dtype=np.float32))
    b = np.asarray(b, dtype=np.float32)
    rec_w, _ = pack_rec_weights([np.asarray(w, dtype=np.float32) for w in Wcs])
    ident = np.eye(128, dtype=np.float32)
    wz = np.zeros((D_IN, 4, 128), dtype=np.float32)
    for m in range(4):
        wz[:, m, 0:64] = W[:, 128 * m : 128 * m + 64]
    in_maps = []
    for c in range(N_CORES):
        m = {
            "X": X[c * B_LOC : (c + 1) * B_LOC],
            "W": W,
            "WZ": wz,
            "RW": rec_w,
            "ID": ident,
        }
        if b_nonzero:
            m["BV"] = b.reshape(1, D_OUT)
        in_maps.append(m)
    return in_maps


def gather(results) -> np.ndarray:
    out = np.empty((B_FULL, D_OUT), dtype=np.float32)
    for c in range(N_CORES):
        o = results[c]["out"]  # (128, 4, B_LOC): unit = 128*chunk + partition
        out[c * B_LOC : (c + 1) * B_LOC] = o.transpose(2, 1, 0).reshape(B_LOC, D_OUT)
    return out


def kernel(X, W, b, Wc0, Wc1, Wc2, Wc3, Wc4, Wc5, Wc6, Wc7) -> np.ndarray:
    Wcs = [Wc0, Wc1, Wc2, Wc3, Wc4, Wc5, Wc6, Wc7]
    b_np = np.asarray(b, dtype=np.float32)
    b_nonzero = bool(np.any(b_np != 0))
    T = int(np.asarray(X).shape[1])
    nc = _get_prog(T, b_nonzero)
    in_maps = make_in_maps(X, W, b_np, Wcs, b_nonzero)
    res = run_bass_kernel_spmd(nc, in_maps, core_ids=list(range(N_CORES)))
    return gather(res.results)
